# revision 18
# baseline (speedup 1.0000x reference)
# Trainium2 Bass kernel for nn_NonLocalBlock (non-local attention block).
#
# Math (per batch sample b):
#   xpe = x[b] + pe[:, :64, :64]                    [192, 4096]
#   q/k/v = W @ xpe + b                             [96, 4096]
#   S[i,j] = (q[:,i] . k[:,j]) / 64                 [4096, 4096]
#   P = exp(S)  (no max subtraction: |S| <= ~1.2)
#   z[:,i] = (v @ P[i,:]) / sum_j P[i,j]
#   out = xpe + Wz @ z + bz                         [192, 4096]
#
# Sharding: data-parallel, one sample per NeuronCore (n=8 == 8 cores).
#
# Device-side layout (everything "transposed" so no on-chip transposes needed):
#   - S is computed as S^T tiles [j_block=128, i_chunk] via matmul
#     lhsT=k[:, jb] (K=96, M=128), rhs=q (K=96, N=i_chunk).
#   - exp via ScalarE ACTIVATE (scale=1/64 folded in), PSUM -> SBUF bf16.
#   - z accumulated as lhsT=vT_aug[jb] (K=128, M=97) @ P^T tiles; vT is
#     computed directly in transposed layout (lhsT=xpe[:, jb], rhs=wvT).
#     vT is augmented with a ones column so row 96 of the accumulator is
#     the softmax denominator.
#   - biases folded into the matmuls via a ones row appended to xpe/znorm
#     and a bias row appended to the (pre-transposed) weights.

import numpy as np
import ml_dtypes

import concourse.bass as bass
import concourse.tile as tile
from concourse import bacc, mybir
from concourse.bass_utils import run_bass_kernel_spmd

F32 = mybir.dt.float32
BF16 = mybir.dt.bfloat16
EXP = mybir.ActivationFunctionType.Exp

NCH = 192   # input/output channels
C = 96      # qkv channels = NCH // 2
HW = 4096   # 64*64 spatial
NB = 8      # batch == number of cores

IC = 1024          # i-chunk (attention query columns per chunk)
NIC = HW // IC     # 4
JB = 128           # j-block (key rows per S^T tile)
NJB = HW // JB     # 32


def _emit(tc):
    nc = tc.nc

    xb = nc.dram_tensor("xb", [NCH, HW], F32, kind="ExternalInput").ap()
    peb = nc.dram_tensor("peb", [NCH, HW], F32, kind="ExternalInput").ap()
    xb_bf = nc.dram_tensor("xb_bf", [NCH, HW], BF16, kind="ExternalInput").ap()
    peb_bf = nc.dram_tensor("peb_bf", [NCH, HW], BF16, kind="ExternalInput").ap()
    wqT = nc.dram_tensor("wqT", [NCH + 1, C], BF16, kind="ExternalInput").ap()
    wkT = nc.dram_tensor("wkT", [NCH + 1, C], BF16, kind="ExternalInput").ap()
    wvT = nc.dram_tensor("wvT", [NCH + 1, C], BF16, kind="ExternalInput").ap()
    wzT = nc.dram_tensor("wzT", [C + 1, NCH], BF16, kind="ExternalInput").ap()
    yb = nc.dram_tensor("yb", [NCH, HW], F32, kind="ExternalOutput").ap()

    const = tc.alloc_tile_pool(name="const", bufs=1)
    ld = tc.alloc_tile_pool(name="ld", bufs=2)
    pp = tc.alloc_tile_pool(name="pp", bufs=4, space="PSUM")

    # ---- persistent SBUF tiles ----
    xpe_hi = const.tile([128, HW], F32)       # channels 0..127 of x+pe
    xpe_lo = const.tile([65, HW], F32)        # channels 128..191; row 64 = 1.0
    xbf_hi = const.tile([128, HW], BF16)
    xbf_lo = const.tile([65, HW], BF16)
    q_sb = const.tile([C, HW], BF16)
    k_sb = const.tile([C, HW], BF16)
    vT_sb = const.tile([128, NJB * (C + 1)], BF16)   # per jb: [128, 97], col 96 = 1.0
    zn_sb = const.tile([C + 1, HW], BF16)            # normalized z; row 96 = 1.0

    wq_hi = const.tile([128, C], BF16)
    wq_lo = const.tile([65, C], BF16)
    wk_hi = const.tile([128, C], BF16)
    wk_lo = const.tile([65, C], BF16)
    wv_hi = const.tile([128, C], BF16)
    wv_lo = const.tile([65, C], BF16)
    wz_sb = const.tile([C + 1, NCH], BF16)

    # ---- weight loads + constant rows ----
    nc.sync.dma_start(out=wq_hi, in_=wqT[0:128, :])
    nc.sync.dma_start(out=wq_lo, in_=wqT[128 : NCH + 1, :])
    nc.sync.dma_start(out=wk_hi, in_=wkT[0:128, :])
    nc.sync.dma_start(out=wk_lo, in_=wkT[128 : NCH + 1, :])
    nc.sync.dma_start(out=wv_hi, in_=wvT[0:128, :])
    nc.sync.dma_start(out=wv_lo, in_=wvT[128 : NCH + 1, :])
    nc.sync.dma_start(out=wz_sb, in_=wzT)

    nc.gpsimd.memset(vT_sb, 1.0)
    nc.gpsimd.memset(zn_sb[C : C + 1, :], 1.0)
    nc.gpsimd.memset(xpe_lo[64:65, :], 1.0)
    nc.gpsimd.memset(xbf_lo[64:65, :], 1.0)

    # pull the ACT exp table load (~2.7us) to t=0
    dummy = const.tile([1, 1], F32)
    nc.vector.memset(dummy, 0.0)
    nc.scalar.activation(dummy, dummy, EXP)

    # ---- bf16 x/pe load + add first: the whole kernel start hangs on these ----
    for t in range(HW // 2048):
        cs = bass.ts(t, 2048)
        peh = ld.tile([128, 2048], BF16, tag="peh")
        nc.sync.dma_start(out=xbf_hi[:, cs], in_=xb_bf[0:128, cs])
        nc.sync.dma_start(out=peh, in_=peb_bf[0:128, cs])
        nc.vector.tensor_add(xbf_hi[:, cs], xbf_hi[:, cs], peh)

        pehl = ld.tile([64, 2048], BF16, tag="pehl")
        nc.sync.dma_start(out=xbf_lo[0:64, cs], in_=xb_bf[128:NCH, cs])
        nc.sync.dma_start(out=pehl, in_=peb_bf[128:NCH, cs])
        nc.vector.tensor_add(xbf_lo[0:64, cs], xbf_lo[0:64, cs], pehl)

    # ---- q/k projections: [96, 4096] bf16 ----
    for t in range(HW // 512):
        cs = bass.ts(t, 512)
        kp = pp.tile([C, 512], F32, tag="p")
        nc.tensor.matmul(kp, wk_hi, xbf_hi[:, cs], start=True, stop=False)
        nc.tensor.matmul(kp, wk_lo, xbf_lo[:, cs], start=False, stop=True)
        nc.vector.tensor_copy(k_sb[:, cs], kp)

        qp = pp.tile([C, 512], F32, tag="p")
        nc.tensor.matmul(qp, wq_hi, xbf_hi[:, cs], start=True, stop=False)
        nc.tensor.matmul(qp, wq_lo, xbf_lo[:, cs], start=False, stop=True)
        nc.vector.tensor_copy(q_sb[:, cs], qp)

    # ---- f32 x/pe for the residual path: not needed until the output
    # projection, so loaded at lower priority (overlaps attention) ----
    for t in range(HW // 2048):
        cs = bass.ts(t, 2048)
        pef = ld.tile([128, 2048], F32, tag="pef")
        nc.sync.dma_start(out=xpe_hi[:, cs], in_=xb[0:128, cs])
        nc.sync.dma_start(out=pef, in_=peb[0:128, cs])
        nc.vector.tensor_add(xpe_hi[:, cs], xpe_hi[:, cs], pef)

        pefl = ld.tile([64, 2048], F32, tag="pefl")
        nc.sync.dma_start(out=xpe_lo[0:64, cs], in_=xb[128:NCH, cs])
        nc.sync.dma_start(out=pefl, in_=peb[128:NCH, cs])
        nc.vector.tensor_add(xpe_lo[0:64, cs], xpe_lo[0:64, cs], pefl)

    pp.release()

    # ---- attention pools (PSUM: 2x2 st + 1x2 zacc + 2x1 zo = 8 banks) ----
    stp = tc.alloc_tile_pool(name="stp", bufs=2, space="PSUM")
    zap = tc.alloc_tile_pool(name="zap", bufs=1, space="PSUM")
    zop = tc.alloc_tile_pool(name="zop", bufs=2, space="PSUM")

    def emit_vt(jb):
        # vT_aug[jb] = [128 (j), 96 (c)]: lhsT = xpe[:, jb block] (stationary),
        # rhs = wvT -> out[j, c] = sum_ch xpe[ch, j] wv[c, ch].
        # Borrows a "zo" PSUM slot (zo is idle during the first i-chunk).
        js = bass.ts(jb, JB)
        vp = zop.tile([JB, C], F32, tag="zo")
        nc.tensor.matmul(vp, xbf_hi[:, js], wv_hi, start=True, stop=False)
        nc.tensor.matmul(vp, xbf_lo[:, js], wv_lo, start=False, stop=True)
        nc.vector.tensor_copy(vT_sb[:, jb * (C + 1) : jb * (C + 1) + C], vp)
    ptp = tc.alloc_tile_pool(name="ptp", bufs=6)
    npool = tc.alloc_tile_pool(name="npool", bufs=2)
    outp = tc.alloc_tile_pool(name="outp", bufs=3)
    dramp = tc.alloc_tile_pool(name="dramp", bufs=2, space="DRAM")

    for ic in range(NIC):
        ics = bass.ts(ic, IC)
        zacc = zap.tile([C + 1, IC], F32, tag="zacc")
        for jb in range(NJB):
            js = bass.ts(jb, JB)
            st = stp.tile([JB, IC], F32, tag="st")
            for h in range(IC // 512):
                nc.tensor.matmul(
                    st[:, bass.ts(h, 512)],
                    k_sb[:, js],
                    q_sb[:, bass.ds(ic * IC + h * 512, 512)],
                    start=True,
                    stop=True,
                )
            pt = ptp.tile([JB, IC], BF16, tag="pt")
            nc.scalar.activation(pt, st, EXP, scale=1.0 / 64.0)
            if ic == 0:
                # interleave vT computation with the first i-chunk so the
                # ScalarE exp pipeline starts early
                emit_vt(jb)
            vts = vT_sb[:, jb * (C + 1) : (jb + 1) * (C + 1)]
            for h in range(IC // 512):
                nc.tensor.matmul(
                    zacc[:, bass.ts(h, 512)],
                    vts,
                    pt[:, bass.ts(h, 512)],
                    start=(jb == 0),
                    stop=(jb == NJB - 1),
                )
        # softmax denominator is row 96 of zacc (ones column of vT_aug)
        recip = npool.tile([1, IC], F32, tag="recip")
        nc.vector.reciprocal(recip, zacc[C : C + 1, :])
        # partition-broadcast needs a DRAM source: bounce through scratch
        rd = dramp.tile([1, IC], F32, tag="rd")
        nc.sync.dma_start(out=rd, in_=recip)
        rb = npool.tile([C, IC], F32, tag="rb")
        nc.sync.dma_start(out=rb, in_=rd.to_broadcast([C, IC]))
        nc.vector.tensor_mul(zn_sb[0:C, ics], zacc[0:C, :], rb)

        # output projection for this i-chunk; PSUM tiles share the "st" slots
        for t in range(IC // 512):
            cs = bass.ds(ic * IC + t * 512, 512)
            zo = zop.tile([128, 512], F32, tag="zo")
            nc.tensor.matmul(zo, wz_sb[:, 0:128], zn_sb[:, cs], start=True, stop=True)
            oh = outp.tile([128, 512], F32, tag="oh")
            nc.vector.tensor_add(oh, zo, xpe_hi[:, cs])
            nc.sync.dma_start(out=yb[0:128, cs], in_=oh)

            zo2 = zop.tile([64, 512], F32, tag="zo")
            nc.tensor.matmul(zo2, wz_sb[:, 128:NCH], zn_sb[:, cs], start=True, stop=True)
            ol = outp.tile([64, 512], F32, tag="ol")
            nc.vector.tensor_add(ol, zo2, xpe_lo[0:64, cs])
            nc.sync.dma_start(out=yb[128:NCH, cs], in_=ol)

    for p in (dramp, outp, npool, ptp, zop, zap, stp, ld, const):
        p.release()


def build_nc():
    nc = bacc.Bacc("TRN2", target_bir_lowering=False, debug=False, num_devices=NB)
    with tile.TileContext(nc) as tc:
        _emit(tc)
    nc.compile()
    return nc


def _prep_inputs(x, pe, wq, bq, wk, bk, wv, bv, wz, bz):
    n, nch, h, w = x.shape
    hw = h * w
    bf = ml_dtypes.bfloat16

    pe_s = np.ascontiguousarray(pe[:, :h, :w]).reshape(nch, hw).astype(np.float32)
    pe_bf = pe_s.astype(bf)
    wqT = np.concatenate([wq.T, bq[None, :]], axis=0).astype(bf)
    wkT = np.concatenate([wk.T, bk[None, :]], axis=0).astype(bf)
    wvT = np.concatenate([wv.T, bv[None, :]], axis=0).astype(bf)
    wzT = np.concatenate([wz.T, bz[None, :]], axis=0).astype(bf)

    xf = np.ascontiguousarray(x.reshape(n, nch, hw)).astype(np.float32)
    in_maps = []
    for b in range(n):
        in_maps.append(
            {
                "xb": xf[b],
                "peb": pe_s,
                "xb_bf": xf[b].astype(bf),
                "peb_bf": pe_bf,
                "wqT": wqT,
                "wkT": wkT,
                "wvT": wvT,
                "wzT": wzT,
            }
        )
    return in_maps


def run(inputs, trace=False):
    in_maps = _prep_inputs(**inputs)
    nc = build_nc()
    res = run_bass_kernel_spmd(
        nc, in_maps, core_ids=list(range(NB)), trace=trace
    )
    n, nch, h, w = inputs["x"].shape
    out = np.stack([r["yb"] for r in res.results]).reshape(n, nch, h, w)
    return out, res


def kernel(**inputs):
    out, _ = run(inputs, trace=False)
    return out


# revision 22
# speedup vs baseline: 21.2989x; 21.2989x over previous
# Trainium2 Bass kernel for nn_NonLocalBlock (non-local attention block).
#
# Math (per batch sample b):
#   xpe = x[b] + pe[:, :64, :64]                    [192, 4096]
#   q/k/v = W @ xpe + b                             [96, 4096]
#   S[i,j] = (q[:,i] . k[:,j]) / 64                 [4096, 4096]
#   P = exp(S)  (no max subtraction: |S| <= ~1.2)
#   z[:,i] = (v @ P[i,:]) / sum_j P[i,j]
#   out = xpe + Wz @ z + bz                         [192, 4096]
#
# Sharding: data-parallel, one sample per NeuronCore (n=8 == 8 cores).
#
# Device-side layout (everything "transposed" so no on-chip transposes needed):
#   - S is computed as S^T tiles [j_block=128, i_chunk] via matmul
#     lhsT=k[:, jb] (K=96, M=128), rhs=q (K=96, N=i_chunk).
#   - exp via ScalarE ACTIVATE (scale=1/64 folded in), PSUM -> SBUF bf16.
#   - z accumulated as lhsT=vT_aug[jb] (K=128, M=97) @ P^T tiles; vT is
#     computed directly in transposed layout (lhsT=xpe[:, jb], rhs=wvT).
#     vT is augmented with a ones column so row 96 of the accumulator is
#     the softmax denominator.
#   - biases folded into the matmuls via a ones row appended to xpe/znorm
#     and a bias row appended to the (pre-transposed) weights.

import numpy as np
import ml_dtypes

import concourse.bass as bass
import concourse.tile as tile
from concourse import bacc, mybir
from concourse.bass_utils import run_bass_kernel_spmd

F32 = mybir.dt.float32
BF16 = mybir.dt.bfloat16
EXP = mybir.ActivationFunctionType.Exp

NCH = 192   # input/output channels
C = 96      # qkv channels = NCH // 2
HW = 4096   # 64*64 spatial
NB = 8      # batch == number of cores

IC = 1024          # i-chunk (attention query columns per chunk)
NIC = HW // IC     # 4
JB = 128           # j-block (key rows per S^T tile)
NJB = HW // JB     # 32


def _emit(tc, reps=1):
    nc = tc.nc

    xb = nc.dram_tensor("xb", [NCH, HW], F32, kind="ExternalInput").ap()
    peb = nc.dram_tensor("peb", [NCH, HW], F32, kind="ExternalInput").ap()
    xb_bf = nc.dram_tensor("xb_bf", [NCH, HW], BF16, kind="ExternalInput").ap()
    peb_bf = nc.dram_tensor("peb_bf", [NCH, HW], BF16, kind="ExternalInput").ap()
    wqT = nc.dram_tensor("wqT", [NCH + 1, C], BF16, kind="ExternalInput").ap()
    wkT = nc.dram_tensor("wkT", [NCH + 1, C], BF16, kind="ExternalInput").ap()
    wvT = nc.dram_tensor("wvT", [NCH + 1, C], BF16, kind="ExternalInput").ap()
    wzT = nc.dram_tensor("wzT", [C + 1, NCH], BF16, kind="ExternalInput").ap()
    yb = nc.dram_tensor("yb", [NCH, HW], F32, kind="ExternalOutput").ap()

    const = tc.alloc_tile_pool(name="const", bufs=1)
    ld = tc.alloc_tile_pool(name="ld", bufs=2)

    # ---- persistent SBUF tiles ----
    xpe_hi = const.tile([128, HW], F32)       # channels 0..127 of x+pe
    xpe_lo = const.tile([65, HW], F32)        # channels 128..191; row 64 = 1.0
    xbf_hi = const.tile([128, HW], BF16)
    xbf_lo = const.tile([65, HW], BF16)
    q_sb = const.tile([C, HW], BF16)
    k_sb = const.tile([C, HW], BF16)
    vT_sb = const.tile([128, NJB * (C + 1)], BF16)   # per jb: [128, 97], col 96 = 1.0
    zn_sb = const.tile([C + 1, HW], BF16)            # normalized z; row 96 = 1.0

    wq_hi = const.tile([128, C], BF16)
    wq_lo = const.tile([65, C], BF16)
    wk_hi = const.tile([128, C], BF16)
    wk_lo = const.tile([65, C], BF16)
    wv_hi = const.tile([128, C], BF16)
    wv_lo = const.tile([65, C], BF16)
    wz_sb = const.tile([C + 1, NCH], BF16)

    # ---- weight loads + constant rows ----
    nc.sync.dma_start(out=wq_hi, in_=wqT[0:128, :])
    nc.sync.dma_start(out=wq_lo, in_=wqT[128 : NCH + 1, :])
    nc.sync.dma_start(out=wk_hi, in_=wkT[0:128, :])
    nc.sync.dma_start(out=wk_lo, in_=wkT[128 : NCH + 1, :])
    nc.sync.dma_start(out=wv_hi, in_=wvT[0:128, :])
    nc.sync.dma_start(out=wv_lo, in_=wvT[128 : NCH + 1, :])
    nc.sync.dma_start(out=wz_sb, in_=wzT)

    nc.gpsimd.memset(vT_sb, 1.0)
    nc.gpsimd.memset(zn_sb[C : C + 1, :], 1.0)
    nc.gpsimd.memset(xpe_lo[64:65, :], 1.0)
    nc.gpsimd.memset(xbf_lo[64:65, :], 1.0)

    # pull the ACT exp table load (~2.7us) to t=0
    dummy = const.tile([1, 1], F32)
    nc.vector.memset(dummy, 0.0)
    nc.scalar.activation(dummy, dummy, EXP)

    ptp = tc.alloc_tile_pool(name="ptp", bufs=6)
    npool = tc.alloc_tile_pool(name="npool", bufs=2)
    outp = tc.alloc_tile_pool(name="outp", bufs=3)
    dramp = tc.alloc_tile_pool(name="dramp", bufs=2, space="DRAM")

    for rep in range(reps):
        if rep > 0:
            # reps>1 is a benchmarking configuration: serialize reps so the
            # wall-time slope measures one full single-shot body
            tc.strict_bb_all_engine_barrier()

        pp = tc.alloc_tile_pool(name="pp", bufs=4, space="PSUM")

        # ---- bf16 x/pe load + add first: kernel start hangs on these ----
        for t in range(HW // 2048):
            cs = bass.ts(t, 2048)
            peh = ld.tile([128, 2048], BF16, tag="peh")
            nc.sync.dma_start(out=xbf_hi[:, cs], in_=xb_bf[0:128, cs])
            nc.sync.dma_start(out=peh, in_=peb_bf[0:128, cs])
            nc.vector.tensor_add(xbf_hi[:, cs], xbf_hi[:, cs], peh)

            pehl = ld.tile([64, 2048], BF16, tag="pehl")
            nc.sync.dma_start(out=xbf_lo[0:64, cs], in_=xb_bf[128:NCH, cs])
            nc.sync.dma_start(out=pehl, in_=peb_bf[128:NCH, cs])
            nc.vector.tensor_add(xbf_lo[0:64, cs], xbf_lo[0:64, cs], pehl)

        # ---- q/k projections: [96, 4096] bf16 ----
        for t in range(HW // 512):
            cs = bass.ts(t, 512)
            kp = pp.tile([C, 512], F32, tag="p")
            nc.tensor.matmul(kp, wk_hi, xbf_hi[:, cs], start=True, stop=False)
            nc.tensor.matmul(kp, wk_lo, xbf_lo[:, cs], start=False, stop=True)
            nc.vector.tensor_copy(k_sb[:, cs], kp)

            qp = pp.tile([C, 512], F32, tag="p")
            nc.tensor.matmul(qp, wq_hi, xbf_hi[:, cs], start=True, stop=False)
            nc.tensor.matmul(qp, wq_lo, xbf_lo[:, cs], start=False, stop=True)
            nc.vector.tensor_copy(q_sb[:, cs], qp)

        # ---- f32 x/pe for the residual path: not needed until the output
        # projection, so loaded at lower priority (overlaps attention) ----
        for t in range(HW // 2048):
            cs = bass.ts(t, 2048)
            pef = ld.tile([128, 2048], F32, tag="pef")
            nc.sync.dma_start(out=xpe_hi[:, cs], in_=xb[0:128, cs])
            nc.sync.dma_start(out=pef, in_=peb[0:128, cs])
            nc.vector.tensor_add(xpe_hi[:, cs], xpe_hi[:, cs], pef)

            pefl = ld.tile([64, 2048], F32, tag="pefl")
            nc.sync.dma_start(out=xpe_lo[0:64, cs], in_=xb[128:NCH, cs])
            nc.sync.dma_start(out=pefl, in_=peb[128:NCH, cs])
            nc.vector.tensor_add(xpe_lo[0:64, cs], xpe_lo[0:64, cs], pefl)

        pp.release()

        # ---- attention pools (PSUM: 2x2 st + 1x2 zacc + 2x1 zo = 8 banks) ----
        stp = tc.alloc_tile_pool(name="stp", bufs=2, space="PSUM")
        zap = tc.alloc_tile_pool(name="zap", bufs=1, space="PSUM")
        zop = tc.alloc_tile_pool(name="zop", bufs=2, space="PSUM")

        def emit_vt(jb):
            # vT_aug[jb] = [128 (j), 96 (c)]: lhsT = xpe[:, jb block]
            # (stationary), rhs = wvT -> out[j, c] = sum_ch xpe[ch, j] wv[c, ch].
            # Borrows a "zo" PSUM slot (zo is idle during the first i-chunk).
            js = bass.ts(jb, JB)
            vp = zop.tile([JB, C], F32, tag="zo")
            nc.tensor.matmul(vp, xbf_hi[:, js], wv_hi, start=True, stop=False)
            nc.tensor.matmul(vp, xbf_lo[:, js], wv_lo, start=False, stop=True)
            nc.vector.tensor_copy(vT_sb[:, jb * (C + 1) : jb * (C + 1) + C], vp)

        for ic in range(NIC):
            ics = bass.ts(ic, IC)
            zacc = zap.tile([C + 1, IC], F32, tag="zacc")
            for jb in range(NJB):
                js = bass.ts(jb, JB)
                st = stp.tile([JB, IC], F32, tag="st")
                for h in range(IC // 512):
                    nc.tensor.matmul(
                        st[:, bass.ts(h, 512)],
                        k_sb[:, js],
                        q_sb[:, bass.ds(ic * IC + h * 512, 512)],
                        start=True,
                        stop=True,
                    )
                pt = ptp.tile([JB, IC], BF16, tag="pt")
                nc.scalar.activation(pt, st, EXP, scale=1.0 / 64.0)
                if ic == 0:
                    # interleave vT computation with the first i-chunk so the
                    # ScalarE exp pipeline starts early
                    emit_vt(jb)
                vts = vT_sb[:, jb * (C + 1) : (jb + 1) * (C + 1)]
                for h in range(IC // 512):
                    nc.tensor.matmul(
                        zacc[:, bass.ts(h, 512)],
                        vts,
                        pt[:, bass.ts(h, 512)],
                        start=(jb == 0),
                        stop=(jb == NJB - 1),
                    )
            # softmax denominator is row 96 of zacc (ones column of vT_aug)
            recip = npool.tile([1, IC], F32, tag="recip")
            nc.vector.reciprocal(recip, zacc[C : C + 1, :])
            # partition-broadcast needs a DRAM source: bounce through scratch
            rd = dramp.tile([1, IC], F32, tag="rd")
            nc.sync.dma_start(out=rd, in_=recip)
            rb = npool.tile([C, IC], F32, tag="rb")
            nc.sync.dma_start(out=rb, in_=rd.to_broadcast([C, IC]))
            nc.vector.tensor_mul(zn_sb[0:C, ics], zacc[0:C, :], rb)

            # output projection for this i-chunk
            for t in range(IC // 512):
                cs = bass.ds(ic * IC + t * 512, 512)
                zo = zop.tile([128, 512], F32, tag="zo")
                nc.tensor.matmul(
                    zo, wz_sb[:, 0:128], zn_sb[:, cs], start=True, stop=True
                )
                oh = outp.tile([128, 512], F32, tag="oh")
                nc.vector.tensor_add(oh, zo, xpe_hi[:, cs])
                nc.sync.dma_start(out=yb[0:128, cs], in_=oh)

                zo2 = zop.tile([64, 512], F32, tag="zo")
                nc.tensor.matmul(
                    zo2, wz_sb[:, 128:NCH], zn_sb[:, cs], start=True, stop=True
                )
                ol = outp.tile([64, 512], F32, tag="ol")
                nc.vector.tensor_add(ol, zo2, xpe_lo[0:64, cs])
                nc.sync.dma_start(out=yb[128:NCH, cs], in_=ol)

        for p in (zop, zap, stp):
            p.release()

    for p in (dramp, outp, npool, ptp, ld, const):
        p.release()


def build_nc(reps=1):
    nc = bacc.Bacc("TRN2", target_bir_lowering=False, debug=False, num_devices=NB)
    with tile.TileContext(nc) as tc:
        _emit(tc, reps=reps)
    nc.compile()
    return nc


def _prep_inputs(x, pe, wq, bq, wk, bk, wv, bv, wz, bz):
    n, nch, h, w = x.shape
    hw = h * w
    bf = ml_dtypes.bfloat16

    pe_s = np.ascontiguousarray(pe[:, :h, :w]).reshape(nch, hw).astype(np.float32)
    pe_bf = pe_s.astype(bf)
    wqT = np.concatenate([wq.T, bq[None, :]], axis=0).astype(bf)
    wkT = np.concatenate([wk.T, bk[None, :]], axis=0).astype(bf)
    wvT = np.concatenate([wv.T, bv[None, :]], axis=0).astype(bf)
    wzT = np.concatenate([wz.T, bz[None, :]], axis=0).astype(bf)

    xf = np.ascontiguousarray(x.reshape(n, nch, hw)).astype(np.float32)
    in_maps = []
    for b in range(n):
        in_maps.append(
            {
                "xb": xf[b],
                "peb": pe_s,
                "xb_bf": xf[b].astype(bf),
                "peb_bf": pe_bf,
                "wqT": wqT,
                "wkT": wkT,
                "wvT": wvT,
                "wzT": wzT,
            }
        )
    return in_maps


def run(inputs, trace=False):
    in_maps = _prep_inputs(**inputs)
    nc = build_nc()
    res = run_bass_kernel_spmd(
        nc, in_maps, core_ids=list(range(NB)), trace=trace
    )
    n, nch, h, w = inputs["x"].shape
    out = np.stack([r["yb"] for r in res.results]).reshape(n, nch, h, w)
    return out, res


def kernel(**inputs):
    out, _ = run(inputs, trace=False)
    return out
